# revision 14
# baseline (speedup 1.0000x reference)
"""Trainium2 Bass kernel for nn_BaseLinearSSM.

y[b,t] = Re(C @ x_{t+1}) + D @ u[b,t] + bias,  x_{t+1} = A x_t + B u_t  (complex A,B,C)

Strategy:
  Host (fp64): eigendecompose A = V diag(w) V^-1  (cond(V) ~ 370 for this
  problem class), fold V into B/C:  Bt = V^-1 B, Ct = C V.  The recurrence
  becomes diagonal:  xt_{t+1} = w * xt_t + Bt u_t.  Writing w = rho*e^{i th},
  z_t = e^{-i th t} xt_t obeys  z_t = rho * z_{t-1} + e^{-i th t} (Bt u)_t --
  two *real* first-order scans per mode, which map 1:1 onto the DVE's native
  tensor_tensor_scan (state = data0*state + data1).

  Device (per core, batch-sharded 2 of 16):
    f = Bt^T-matmuls of u  ->  modulate by cos/sin(th*t) tables (host fp64)
    -> tensor_tensor_scan along t  ->  demodulate  ->  y = CtRe.x_r - CtIm.x_i
    + D u accumulated in one PSUM group.

  Cores are fully independent (A/B/C/D replicated); host shards u and
  gathers y.
"""

import sys

import numpy as np

if "/opt/trn_rl_repo" not in sys.path:
    sys.path.insert(0, "/opt/trn_rl_repo")

BATCH, T, IN, OUT, N = 16, 2048, 128, 128, 512
NCORES = 8
BLOCAL = BATCH // NCORES  # 2
COLS = BLOCAL * T         # 4096 columns per core, col = b*T + t
NT = N // 128             # 4 partition tiles over the state dim
BLK = 512                 # columns per pipeline block
NBLK = COLS // BLK        # 8 blocks, (b, tb) with tb in 0..3
TBLK = T // BLK           # 4 t-blocks per batch element
# blob layout: ut | btr | bti | dwt | ctr | cti | rho*NT | per-tb (cos*NT | sin*NT)
P0W = COLS + N + N + OUT + NT * OUT + NT * OUT + NT * BLK
TBW = 2 * NT * BLK  # one tb's cos+sin tables
BLOBW = P0W + TBLK * TBW

LAST_RESULT = None  # BassKernelResults of the most recent run (for profiling)

_NC_CACHE = None


def _build_nc():
    """Build the SPMD Bass program (identical on all 8 cores)."""
    from concourse import bass, mybir
    from concourse import tile

    dt = mybir.dt.float32
    op = mybir.AluOpType

    nc = bass.Bass("TRN2", target_bir_lowering=False, debug=False)

    # All inputs packed in ONE [128, W] blob -> one DMA -> one HW queue ->
    # at most one DMA sync wait on any consumer (fused fp32 LDW+MATMUL
    # supports a single sync wait).
    blob = nc.dram_tensor("blob", [128, BLOBW], dt, kind="ExternalInput")
    yout = nc.dram_tensor("y", [OUT, COLS], dt, kind="ExternalOutput")  # [o, b*T+t]

    with tile.TileContext(nc) as tc:
        with (
            tc.tile_pool(name="const", bufs=1) as cpool,
            tc.tile_pool(name="tmp", bufs=3) as tpool,
            tc.tile_pool(name="gp", bufs=1) as gpool,
            tc.tile_pool(name="zp", bufs=2) as zpool,
            tc.tile_pool(name="xp", bufs=1) as xpool,
            tc.tile_pool(name="ysb", bufs=8) as spool,
            tc.tile_pool(name="fps", bufs=6, space="PSUM") as fpool,
            tc.tile_pool(name="yps", bufs=2, space="PSUM") as ypool,
        ):
            blob_sb = cpool.tile([128, BLOBW], dt)
            # Piece 0 (ut + weights + rho) lands first so PE can start; the
            # per-tb table pieces stream in behind it.  _legalize_multi_waits
            # keeps any resulting wait pairing legal for walrus.
            nc.sync.dma_start(blob_sb[:, 0:P0W], blob[:, 0:P0W])
            for k in range(TBLK):
                nc.sync.dma_start(
                    blob_sb[:, P0W + k * TBW:P0W + (k + 1) * TBW],
                    blob[:, P0W + k * TBW:P0W + (k + 1) * TBW],
                )
            o = [0]
            def take(w):
                s = blob_sb[:, o[0]:o[0] + w]
                o[0] += w
                return s
            ut_sb = take(COLS)
            btr_sb = take(N)
            bti_sb = take(N)
            dwt_sb = take(OUT)
            ctr_sb = take(NT * OUT)
            cti_sb = take(NT * OUT)
            rho_sb = [take(BLK) for _ in range(NT)]
            # ct_tb[tb][m] / st_tb[tb][m]: [128, BLK] table slices
            ct_tb = [[None] * NT for _ in range(TBLK)]
            st_tb = [[None] * NT for _ in range(TBLK)]
            for k in range(TBLK):
                for m in range(NT):
                    ct_tb[k][m] = take(BLK)
                for m in range(NT):
                    st_tb[k][m] = take(BLK)
            assert o[0] == BLOBW

            zr_prev = [None] * NT
            zi_prev = [None] * NT
            for b in range(BLOCAL):
                for tb in range(TBLK):
                    col0 = b * T + tb * BLK
                    ucols = ut_sb[:, col0:col0 + BLK]
                    xr_blk = [None] * NT
                    xi_blk = [None] * NT
                    for m in range(NT):
                        ctt = ct_tb[tb][m][:]
                        stt = st_tb[tb][m][:]
                        # f = Bt u  (complex), PSUM
                        fre = fpool.tile([128, BLK], dt, tag="f")
                        fim = fpool.tile([128, BLK], dt, tag="f")
                        nc.tensor.matmul(
                            fre[:], btr_sb[:, m * 128:(m + 1) * 128], ucols
                        )
                        nc.tensor.matmul(
                            fim[:], bti_sb[:, m * 128:(m + 1) * 128], ucols
                        )
                        # modulate: g = e^{-i th t} f
                        t1 = tpool.tile([128, BLK], dt, tag="t1")
                        t2 = tpool.tile([128, BLK], dt, tag="t2")
                        nc.vector.tensor_tensor(t1[:], ctt, fre[:], op=op.mult)
                        nc.vector.tensor_tensor(t2[:], stt, fim[:], op=op.mult)
                        gr = gpool.tile([128, BLK], dt, tag=f"gr{m}")
                        nc.vector.tensor_tensor(gr[:], t1[:], t2[:], op=op.add)
                        t3 = tpool.tile([128, BLK], dt, tag="t1")
                        t4 = tpool.tile([128, BLK], dt, tag="t2")
                        nc.vector.tensor_tensor(t3[:], ctt, fim[:], op=op.mult)
                        nc.vector.tensor_tensor(t4[:], stt, fre[:], op=op.mult)
                        gi = gpool.tile([128, BLK], dt, tag=f"gi{m}")
                        nc.gpsimd.tensor_tensor(gi[:], t3[:], t4[:], op=op.subtract)
                        # scan: z = rho*z_prev + g along t (chained across tb)
                        zr = zpool.tile([128, BLK], dt, tag=f"zr{m}")
                        zi = zpool.tile([128, BLK], dt, tag=f"zi{m}")
                        init_r = 0.0 if tb == 0 else zr_prev[m][:, BLK - 1:BLK]
                        init_i = 0.0 if tb == 0 else zi_prev[m][:, BLK - 1:BLK]
                        nc.vector.tensor_tensor_scan(
                            zr[:], rho_sb[m][:], gr[:], init_r, op0=op.mult, op1=op.add
                        )
                        nc.vector.tensor_tensor_scan(
                            zi[:], rho_sb[m][:], gi[:], init_i, op0=op.mult, op1=op.add
                        )
                        zr_prev[m], zi_prev[m] = zr, zi
                        # demodulate: x = e^{i th t} z
                        t5 = tpool.tile([128, BLK], dt, tag="t1")
                        t6 = tpool.tile([128, BLK], dt, tag="t2")
                        nc.vector.tensor_tensor(t5[:], ctt, zr[:], op=op.mult)
                        nc.vector.tensor_tensor(t6[:], stt, zi[:], op=op.mult)
                        xr = xpool.tile([128, BLK], dt, tag=f"xr{m}")
                        nc.vector.tensor_tensor(xr[:], t5[:], t6[:], op=op.subtract)
                        t7 = tpool.tile([128, BLK], dt, tag="t7")
                        t8 = tpool.tile([128, BLK], dt, tag="t8")
                        nc.gpsimd.tensor_tensor(t7[:], stt, zr[:], op=op.mult)
                        nc.gpsimd.tensor_tensor(t8[:], ctt, zi[:], op=op.mult)
                        xi = xpool.tile([128, BLK], dt, tag=f"xi{m}")
                        nc.gpsimd.tensor_tensor(xi[:], t7[:], t8[:], op=op.add)
                        xr_blk[m], xi_blk[m] = xr, xi
                    # y = sum_m CtRe_m^T x_r[m] + (-CtIm_m)^T x_i[m] + D^T u
                    yps = ypool.tile([128, BLK], dt, tag="y")
                    for m in range(NT):
                        nc.tensor.matmul(
                            yps[:], ctr_sb[:, m * OUT:(m + 1) * OUT], xr_blk[m][:],
                            start=(m == 0), stop=False,
                        )
                        nc.tensor.matmul(
                            yps[:], cti_sb[:, m * OUT:(m + 1) * OUT], xi_blk[m][:],
                            start=False, stop=False,
                        )
                    nc.tensor.matmul(
                        yps[:], dwt_sb[:], ucols, start=False, stop=True
                    )
                    ysb = spool.tile([128, BLK], dt, tag="ysb")
                    nc.scalar.copy(ysb[:], yps[:])
                    nc.gpsimd.dma_start(yout[:, col0:col0 + BLK], ysb[:])

    _legalize_multi_waits(nc)
    return nc


def _legalize_multi_waits(nc):
    """This walrus build accepts a single sync wait per instruction; split
    any multi-wait instruction into same-engine single-wait NoOps + the
    original carrying the last wait (program order chains them)."""
    import bass_rust
    from concourse import mybir

    uid = [0]
    for fn in nc.m.functions:
        for bb in fn.blocks:
            insts = bb.instructions
            new = []
            changed = False
            for inst in insts:
                si = inst.sync_info
                if si is not None and len(si.on_wait) > 1:
                    waits = list(si.on_wait)
                    for w in waits[:-1]:
                        uid[0] += 1
                        new.append(mybir.InstNoOp(
                            name=f"mwsplit-{uid[0]}",
                            engine=inst.engine,
                            ins=[], outs=[],
                            sync_info=bass_rust.SyncInfo(on_wait=[w], on_update=[]),
                        ))
                    inst.sync_info = bass_rust.SyncInfo(
                        on_wait=[waits[-1]], on_update=list(si.on_update)
                    )
                    changed = True
                new.append(inst)
            if changed:
                bb.instructions = new


def _host_prep(A_re, A_im, B_re, B_im, C_re, C_im, D_w):
    """fp64 eigendecomposition + transposed/modulation-table layouts."""
    A = A_re.astype(np.float64) + 1j * A_im.astype(np.float64)
    w, V = np.linalg.eig(A)
    Vinv = np.linalg.inv(V)
    Bt = Vinv @ (B_re.astype(np.float64) + 1j * B_im.astype(np.float64))  # [N, IN]
    Ct = (C_re.astype(np.float64) + 1j * C_im.astype(np.float64)) @ V     # [OUT, N]

    rho = np.abs(w)
    theta = np.angle(w)
    tg = np.arange(1, T + 1, dtype=np.float64)
    ang = np.outer(theta, tg)  # [N, T]
    cost = np.cos(ang).astype(np.float32).reshape(NT, 128, T)
    sint = np.sin(ang).astype(np.float32).reshape(NT, 128, T)
    rho_b = np.broadcast_to(
        rho.astype(np.float32).reshape(NT, 128, 1), (NT, 128, BLK)
    ).copy()

    ctrT = np.ascontiguousarray(Ct.real.T, dtype=np.float32)   # [N, OUT]
    ctiT = np.ascontiguousarray(-Ct.imag.T, dtype=np.float32)  # [N, OUT]
    # shared blob columns (everything except the leading per-core ut block),
    # all [128, w]:
    parts = [
        np.ascontiguousarray(Bt.real.T, dtype=np.float32),  # [128(i), N]
        np.ascontiguousarray(Bt.imag.T, dtype=np.float32),
        np.ascontiguousarray(D_w.T, dtype=np.float32),      # [128(i), OUT]
    ]
    parts += [np.ascontiguousarray(ctrT.reshape(NT, 128, OUT).transpose(1, 0, 2)
                                   .reshape(128, NT * OUT))]
    parts += [np.ascontiguousarray(ctiT.reshape(NT, 128, OUT).transpose(1, 0, 2)
                                   .reshape(128, NT * OUT))]
    parts += [np.ascontiguousarray(rho_b.transpose(1, 0, 2).reshape(128, NT * BLK))]
    for k in range(TBLK):
        cs = cost[:, :, k * BLK:(k + 1) * BLK]  # [NT, 128, BLK]
        ss = sint[:, :, k * BLK:(k + 1) * BLK]
        parts += [np.ascontiguousarray(cs.transpose(1, 0, 2).reshape(128, NT * BLK))]
        parts += [np.ascontiguousarray(ss.transpose(1, 0, 2).reshape(128, NT * BLK))]
    return np.concatenate(parts, axis=1)  # [128, BLOBW - COLS]


def _ensure_axon_hooks():
    """Provide antenv.axon_hooks if the image lacks it (needed only for
    trace=True NTFF profiling; run path works without)."""
    import types
    try:
        from antenv import axon_hooks  # noqa: F401
        return
    except ImportError:
        pass
    try:
        import antenv
        mod = types.ModuleType("antenv.axon_hooks")
        _hook = [None]
        mod.set_axon_ntff_profile_hook = lambda h: _hook.__setitem__(0, h)
        mod.get_axon_ntff_profile_hook = lambda: _hook[0]
        sys.modules["antenv.axon_hooks"] = mod
        antenv.axon_hooks = mod
        if "/root/.axon_site" not in sys.path:
            sys.path.insert(0, "/root/.axon_site")
        from trn_agent_boot.trn_boot import _ntff_profile_via_ctypes
        h = _ntff_profile_via_ctypes("/opt/axon/libaxon_pjrt.so")
        if h is not None:
            mod.set_axon_ntff_profile_hook(h)
    except Exception:
        pass


def kernel(u, A_re, A_im, B_re, B_im, C_re, C_im, D_w, output_bias):
    global LAST_RESULT, _NC_CACHE
    from concourse import bass_utils

    _ensure_axon_hooks()

    u = np.asarray(u, dtype=np.float32)
    shared = _host_prep(
        np.asarray(A_re), np.asarray(A_im), np.asarray(B_re), np.asarray(B_im),
        np.asarray(C_re), np.asarray(C_im), np.asarray(D_w)
    )

    if _NC_CACHE is None:
        _NC_CACHE = _build_nc()
    nc = _NC_CACHE

    in_maps = []
    for k in range(NCORES):
        u_pair = u[BLOCAL * k:BLOCAL * (k + 1)]  # [2, T, IN]
        ut = np.ascontiguousarray(
            u_pair.transpose(2, 0, 1).reshape(128, COLS), dtype=np.float32
        )
        in_maps.append({"blob": np.concatenate([ut, shared], axis=1)})

    res = bass_utils.run_bass_kernel_spmd(nc, in_maps, core_ids=list(range(NCORES)))
    LAST_RESULT = res

    y = np.empty((BATCH, T, OUT), dtype=np.float32)
    for k in range(NCORES):
        yd = res.results[k]["y"]  # [OUT, COLS]
        y[BLOCAL * k:BLOCAL * (k + 1)] = (
            yd.reshape(OUT, BLOCAL, T).transpose(1, 2, 0)
        )
    y += np.asarray(output_bias, dtype=np.float32)
    return y


# revision 16
# speedup vs baseline: 1.0665x; 1.0665x over previous
"""Trainium2 Bass kernel for nn_BaseLinearSSM.

y[b,t] = Re(C @ x_{t+1}) + D @ u[b,t] + bias,  x_{t+1} = A x_t + B u_t  (complex A,B,C)

Strategy:
  Host (fp64): eigendecompose A = V diag(w) V^-1  (cond(V) ~ 370 for this
  problem class), fold V into B/C:  Bt = V^-1 B, Ct = C V.  The recurrence
  becomes diagonal:  xt_{t+1} = w * xt_t + Bt u_t.  Writing w = rho*e^{i th},
  z_t = e^{-i th t} xt_t obeys  z_t = rho * z_{t-1} + e^{-i th t} (Bt u)_t --
  two *real* first-order scans per mode, which map 1:1 onto the DVE's native
  tensor_tensor_scan (state = data0*state + data1).

  Device (per core, batch-sharded 2 of 16):
    f = Bt^T-matmuls of u  ->  modulate by cos/sin(th*t) tables (host fp64)
    -> tensor_tensor_scan along t  ->  demodulate  ->  y = CtRe.x_r - CtIm.x_i
    + D u accumulated in one PSUM group.

  Cores are fully independent (A/B/C/D replicated); host shards u and
  gathers y.
"""

import sys

import numpy as np

if "/opt/trn_rl_repo" not in sys.path:
    sys.path.insert(0, "/opt/trn_rl_repo")

BATCH, T, IN, OUT, N = 16, 2048, 128, 128, 512
NCORES = 8
BLOCAL = BATCH // NCORES  # 2
COLS = BLOCAL * T         # 4096 columns per core, col = b*T + t
NT = N // 128             # 4 partition tiles over the state dim
BLK = 512                 # columns per pipeline block
NBLK = COLS // BLK        # 8 blocks, (b, tb) with tb in 0..3
TBLK = T // BLK           # 4 t-blocks per batch element
# blob layout: ut | btr | bti | dwt | ctr | cti | rho*NT | per-tb (cos*NT | sin*NT)
P0W = COLS + N + N + OUT + NT * OUT + NT * OUT + NT * BLK
TBW = 2 * NT * BLK  # one tb's cos+sin tables
BLOBW = P0W + TBLK * TBW

LAST_RESULT = None  # BassKernelResults of the most recent run (for profiling)

_NC_CACHE = None


def _build_nc():
    """Build the SPMD Bass program (identical on all 8 cores)."""
    from concourse import bass, mybir
    from concourse import tile

    dt = mybir.dt.float32
    op = mybir.AluOpType

    nc = bass.Bass("TRN2", target_bir_lowering=False, debug=False)

    # All inputs packed in ONE [128, W] blob -> one DMA -> one HW queue ->
    # at most one DMA sync wait on any consumer (fused fp32 LDW+MATMUL
    # supports a single sync wait).
    blob = nc.dram_tensor("blob", [128, BLOBW], dt, kind="ExternalInput")
    yout = nc.dram_tensor("y", [OUT, COLS], dt, kind="ExternalOutput")  # [o, b*T+t]

    with tile.TileContext(nc) as tc:
        with (
            tc.tile_pool(name="const", bufs=1) as cpool,
            tc.tile_pool(name="tmp", bufs=2) as tpool,
            tc.tile_pool(name="gp", bufs=1) as gpool,
            tc.tile_pool(name="zp", bufs=2) as zpool,
            tc.tile_pool(name="xr", bufs=1) as xrpool,
            tc.tile_pool(name="xi", bufs=2) as xipool,
            tc.tile_pool(name="ysb", bufs=2) as spool,
            tc.tile_pool(name="fps", bufs=6, space="PSUM") as fpool,
            tc.tile_pool(name="yps", bufs=2, space="PSUM") as ypool,
        ):
            blob_sb = cpool.tile([128, BLOBW], dt)
            # Piece 0 (ut + weights + rho) lands first so PE can start; the
            # per-tb table pieces stream in behind it.  _legalize_multi_waits
            # keeps any resulting wait pairing legal for walrus.
            nc.sync.dma_start(blob_sb[:, 0:P0W], blob[:, 0:P0W])
            for k in range(TBLK):
                nc.sync.dma_start(
                    blob_sb[:, P0W + k * TBW:P0W + (k + 1) * TBW],
                    blob[:, P0W + k * TBW:P0W + (k + 1) * TBW],
                )
            o = [0]
            def take(w):
                s = blob_sb[:, o[0]:o[0] + w]
                o[0] += w
                return s
            ut_sb = take(COLS)
            btr_sb = take(N)
            bti_sb = take(N)
            dwt_sb = take(OUT)
            ctr_sb = take(NT * OUT)
            cti_sb = take(NT * OUT)
            rho_sb = [take(BLK) for _ in range(NT)]
            # ct_tb[tb][m] / st_tb[tb][m]: [128, BLK] table slices
            ct_tb = [[None] * NT for _ in range(TBLK)]
            st_tb = [[None] * NT for _ in range(TBLK)]
            for k in range(TBLK):
                for m in range(NT):
                    ct_tb[k][m] = take(BLK)
                for m in range(NT):
                    st_tb[k][m] = take(BLK)
            assert o[0] == BLOBW

            zr_prev = [None] * NT
            zi_prev = [None] * NT
            for b in range(BLOCAL):
                for tb in range(TBLK):
                    col0 = b * T + tb * BLK
                    ucols = ut_sb[:, col0:col0 + BLK]
                    xr_blk = [None] * NT
                    xi_blk = [None] * NT
                    for m in range(NT):
                        ctt = ct_tb[tb][m][:]
                        stt = st_tb[tb][m][:]
                        # f = Bt u  (complex), PSUM
                        fre = fpool.tile([128, BLK], dt, tag="f")
                        fim = fpool.tile([128, BLK], dt, tag="f")
                        nc.tensor.matmul(
                            fre[:], btr_sb[:, m * 128:(m + 1) * 128], ucols
                        )
                        nc.tensor.matmul(
                            fim[:], bti_sb[:, m * 128:(m + 1) * 128], ucols
                        )
                        # modulate: g = e^{-i th t} f
                        t1 = tpool.tile([128, BLK], dt, tag="t1")
                        t2 = tpool.tile([128, BLK], dt, tag="t2")
                        nc.vector.tensor_tensor(t1[:], ctt, fre[:], op=op.mult)
                        nc.vector.tensor_tensor(t2[:], stt, fim[:], op=op.mult)
                        gr = gpool.tile([128, BLK], dt, tag=f"gr{m}")
                        nc.vector.tensor_tensor(gr[:], t1[:], t2[:], op=op.add)
                        t3 = tpool.tile([128, BLK], dt, tag="t1")
                        t4 = tpool.tile([128, BLK], dt, tag="t2")
                        nc.vector.tensor_tensor(t3[:], ctt, fim[:], op=op.mult)
                        nc.vector.tensor_tensor(t4[:], stt, fre[:], op=op.mult)
                        gi = gpool.tile([128, BLK], dt, tag=f"gi{m}")
                        nc.vector.tensor_tensor(gi[:], t3[:], t4[:], op=op.subtract)
                        # scan: z = rho*z_prev + g along t (chained across tb)
                        zr = zpool.tile([128, BLK], dt, tag=f"zr{m}")
                        zi = zpool.tile([128, BLK], dt, tag=f"zi{m}")
                        init_r = 0.0 if tb == 0 else zr_prev[m][:, BLK - 1:BLK]
                        init_i = 0.0 if tb == 0 else zi_prev[m][:, BLK - 1:BLK]
                        nc.vector.tensor_tensor_scan(
                            zr[:], rho_sb[m][:], gr[:], init_r, op0=op.mult, op1=op.add
                        )
                        nc.vector.tensor_tensor_scan(
                            zi[:], rho_sb[m][:], gi[:], init_i, op0=op.mult, op1=op.add
                        )
                        zr_prev[m], zi_prev[m] = zr, zi
                        # demodulate: x = e^{i th t} z
                        t5 = tpool.tile([128, BLK], dt, tag="t1")
                        t6 = tpool.tile([128, BLK], dt, tag="t2")
                        nc.vector.tensor_tensor(t5[:], ctt, zr[:], op=op.mult)
                        nc.vector.tensor_tensor(t6[:], stt, zi[:], op=op.mult)
                        xr = xrpool.tile([128, BLK], dt, tag=f"xr{m}")
                        nc.vector.tensor_tensor(xr[:], t5[:], t6[:], op=op.subtract)
                        t7 = tpool.tile([128, BLK], dt, tag="t7")
                        t8 = tpool.tile([128, BLK], dt, tag="t8")
                        nc.gpsimd.tensor_tensor(t7[:], stt, zr[:], op=op.mult)
                        nc.gpsimd.tensor_tensor(t8[:], ctt, zi[:], op=op.mult)
                        xi = xipool.tile([128, BLK], dt, tag=f"xi{m}")
                        nc.gpsimd.tensor_tensor(xi[:], t7[:], t8[:], op=op.add)
                        xr_blk[m], xi_blk[m] = xr, xi
                    # y = sum_m CtRe_m^T x_r[m] + (-CtIm_m)^T x_i[m] + D^T u
                    yps = ypool.tile([128, BLK], dt, tag="y")
                    for m in range(NT):
                        nc.tensor.matmul(
                            yps[:], ctr_sb[:, m * OUT:(m + 1) * OUT], xr_blk[m][:],
                            start=(m == 0), stop=False,
                        )
                        nc.tensor.matmul(
                            yps[:], cti_sb[:, m * OUT:(m + 1) * OUT], xi_blk[m][:],
                            start=False, stop=False,
                        )
                    nc.tensor.matmul(
                        yps[:], dwt_sb[:], ucols, start=False, stop=True
                    )
                    ysb = spool.tile([128, BLK], dt, tag="ysb")
                    nc.scalar.copy(ysb[:], yps[:])
                    nc.gpsimd.dma_start(yout[:, col0:col0 + BLK], ysb[:])

    _legalize_multi_waits(nc)
    return nc


def _legalize_multi_waits(nc):
    """This walrus build accepts a single sync wait per instruction; split
    any multi-wait instruction into same-engine single-wait NoOps + the
    original carrying the last wait (program order chains them)."""
    import bass_rust
    from concourse import mybir

    uid = [0]
    for fn in nc.m.functions:
        for bb in fn.blocks:
            insts = bb.instructions
            new = []
            changed = False
            for inst in insts:
                si = inst.sync_info
                if si is not None and len(si.on_wait) > 1:
                    waits = list(si.on_wait)
                    for w in waits[:-1]:
                        uid[0] += 1
                        new.append(mybir.InstNoOp(
                            name=f"mwsplit-{uid[0]}",
                            engine=inst.engine,
                            ins=[], outs=[],
                            sync_info=bass_rust.SyncInfo(on_wait=[w], on_update=[]),
                        ))
                    inst.sync_info = bass_rust.SyncInfo(
                        on_wait=[waits[-1]], on_update=list(si.on_update)
                    )
                    changed = True
                new.append(inst)
            if changed:
                bb.instructions = new


def _host_prep(A_re, A_im, B_re, B_im, C_re, C_im, D_w):
    """fp64 eigendecomposition + transposed/modulation-table layouts."""
    A = A_re.astype(np.float64) + 1j * A_im.astype(np.float64)
    w, V = np.linalg.eig(A)
    Vinv = np.linalg.inv(V)
    Bt = Vinv @ (B_re.astype(np.float64) + 1j * B_im.astype(np.float64))  # [N, IN]
    Ct = (C_re.astype(np.float64) + 1j * C_im.astype(np.float64)) @ V     # [OUT, N]

    rho = np.abs(w)
    theta = np.angle(w)
    tg = np.arange(1, T + 1, dtype=np.float64)
    ang = np.outer(theta, tg)  # [N, T]
    cost = np.cos(ang).astype(np.float32).reshape(NT, 128, T)
    sint = np.sin(ang).astype(np.float32).reshape(NT, 128, T)
    rho_b = np.broadcast_to(
        rho.astype(np.float32).reshape(NT, 128, 1), (NT, 128, BLK)
    ).copy()

    ctrT = np.ascontiguousarray(Ct.real.T, dtype=np.float32)   # [N, OUT]
    ctiT = np.ascontiguousarray(-Ct.imag.T, dtype=np.float32)  # [N, OUT]
    # shared blob columns (everything except the leading per-core ut block),
    # all [128, w]:
    parts = [
        np.ascontiguousarray(Bt.real.T, dtype=np.float32),  # [128(i), N]
        np.ascontiguousarray(Bt.imag.T, dtype=np.float32),
        np.ascontiguousarray(D_w.T, dtype=np.float32),      # [128(i), OUT]
    ]
    parts += [np.ascontiguousarray(ctrT.reshape(NT, 128, OUT).transpose(1, 0, 2)
                                   .reshape(128, NT * OUT))]
    parts += [np.ascontiguousarray(ctiT.reshape(NT, 128, OUT).transpose(1, 0, 2)
                                   .reshape(128, NT * OUT))]
    parts += [np.ascontiguousarray(rho_b.transpose(1, 0, 2).reshape(128, NT * BLK))]
    for k in range(TBLK):
        cs = cost[:, :, k * BLK:(k + 1) * BLK]  # [NT, 128, BLK]
        ss = sint[:, :, k * BLK:(k + 1) * BLK]
        parts += [np.ascontiguousarray(cs.transpose(1, 0, 2).reshape(128, NT * BLK))]
        parts += [np.ascontiguousarray(ss.transpose(1, 0, 2).reshape(128, NT * BLK))]
    return np.concatenate(parts, axis=1)  # [128, BLOBW - COLS]


def _ensure_axon_hooks():
    """Provide antenv.axon_hooks if the image lacks it (needed only for
    trace=True NTFF profiling; run path works without)."""
    import types
    try:
        from antenv import axon_hooks  # noqa: F401
        return
    except ImportError:
        pass
    try:
        import antenv
        mod = types.ModuleType("antenv.axon_hooks")
        _hook = [None]
        mod.set_axon_ntff_profile_hook = lambda h: _hook.__setitem__(0, h)
        mod.get_axon_ntff_profile_hook = lambda: _hook[0]
        sys.modules["antenv.axon_hooks"] = mod
        antenv.axon_hooks = mod
        if "/root/.axon_site" not in sys.path:
            sys.path.insert(0, "/root/.axon_site")
        from trn_agent_boot.trn_boot import _ntff_profile_via_ctypes
        h = _ntff_profile_via_ctypes("/opt/axon/libaxon_pjrt.so")
        if h is not None:
            mod.set_axon_ntff_profile_hook(h)
    except Exception:
        pass


def kernel(u, A_re, A_im, B_re, B_im, C_re, C_im, D_w, output_bias):
    global LAST_RESULT, _NC_CACHE
    from concourse import bass_utils

    _ensure_axon_hooks()

    u = np.asarray(u, dtype=np.float32)
    shared = _host_prep(
        np.asarray(A_re), np.asarray(A_im), np.asarray(B_re), np.asarray(B_im),
        np.asarray(C_re), np.asarray(C_im), np.asarray(D_w)
    )

    if _NC_CACHE is None:
        _NC_CACHE = _build_nc()
    nc = _NC_CACHE

    in_maps = []
    for k in range(NCORES):
        u_pair = u[BLOCAL * k:BLOCAL * (k + 1)]  # [2, T, IN]
        ut = np.ascontiguousarray(
            u_pair.transpose(2, 0, 1).reshape(128, COLS), dtype=np.float32
        )
        in_maps.append({"blob": np.concatenate([ut, shared], axis=1)})

    res = bass_utils.run_bass_kernel_spmd(nc, in_maps, core_ids=list(range(NCORES)))
    LAST_RESULT = res

    y = np.empty((BATCH, T, OUT), dtype=np.float32)
    for k in range(NCORES):
        yd = res.results[k]["y"]  # [OUT, COLS]
        y[BLOCAL * k:BLOCAL * (k + 1)] = (
            yd.reshape(OUT, BLOCAL, T).transpose(1, 2, 0)
        )
    y += np.asarray(output_bias, dtype=np.float32)
    return y
